# revision 1
# baseline (speedup 1.0000x reference)
"""RWKV5 (TimeMix chunked linear attention + ChannelMix) block on 8 trn2\nNeuronCores. Pair tensor-parallel sharding: core c -> batch c//2, half c%2\n(heads / FFN columns); intra-pair AllReduce for Wo and kv partials.\nActivations kept channel-major [C, T] throughout.\n"""
import numpy as np
import concourse.bass as bass
import concourse.mybir as mybir
import concourse.tile as tile
from concourse import bacc
from concourse.masks import make_identity

f32 = mybir.dt.float32
bf16 = mybir.dt.bfloat16
AOT = mybir.AluOpType
AFT = mybir.ActivationFunctionType

C = 2048
CH = 1024      # channel half (heads)
HL = 16        # heads per core
S = 64
TC = 512       # chunk len
DFH = 3584
P = 128
NK = C // P    # 16 c-chunks
NP = CH // P   # 8 pair-chunks
NDF = DFH // P # 28
EPS = 1e-5
HS_DIV = float(np.sqrt(S))
GROUPS = [[0, 1], [2, 3], [4, 5], [6, 7]]


def build_nc(T=2048, mmdt=f32):
    NCH = T // TC
    nc = bacc.Bacc("TRN2", target_bir_lowering=False, debug=False, num_devices=8)
    dp = nc.declare_dram_parameter
    xT = dp("xT", [C, T], f32, isOutput=False)
    wr = dp("wr", [C, CH], mmdt, isOutput=False)
    wk = dp("wk", [C, CH], mmdt, isOutput=False)
    wv = dp("wv", [C, CH], mmdt, isOutput=False)
    wo = dp("wo", [CH, C], mmdt, isOutput=False)
    wfk = dp("wfk", [C, DFH], bf16, isOutput=False)
    wfv = dp("wfv", [DFH, C], bf16, isOutput=False)
    wfr = dp("wfr", [C, C], bf16, isOutput=False)
    # per-channel vectors, host-prepped as [128, NK] (chunk k in col k)
    params = {"xT": xT, "wr": wr, "wk": wk, "wv": wv, "wo": wo,
              "wfk": wfk, "wfv": wfv, "wfr": wfr}
    for nm, nn in [("ln1g", NK), ("ln1b", NK), ("ln2g", NK), ("ln2b", NK),
                   ("mxk", NK), ("mxv", NK), ("mxr", NK), ("fmk", NK),
                   ("fmr", NK), ("lnxg", NP), ("lnxb", NP)]:
        params[nm] = dp(nm, [P, nn], f32, isOutput=False)
    params["tdv"] = dp("tdv", [1, HL], f32, isOutput=False)
    params["uv"] = dp("uv", [1, HL], f32, isOutput=False)
    params["yT"] = dp("yT", [C, T], f32, isOutput=True)
    with tile.TileContext(nc) as tc:
        _build(nc, tc, T, NCH, mmdt, params)
    nc.compile()
    return nc


def _build(nc, tc, T, NCH, mmdt, params):
    def dparam(nm):
        return params[nm]

    ctxs = []
    def pool(name, bufs, space="SBUF"):
        p = tc.tile_pool(name=name, bufs=bufs, space=space)
        ctxs.append(p)
        return p.__enter__()

    const = pool("const", 1)
    pers = pool("pers", 1)
    mega = pool("mega", 62)       # shared 2KB slabs (tag s2)
    scr = pool("scr", 14)         # shared scratch (tag scr)
    wts = pool("wts", 4)          # weight streaming (4KB slots)
    ps = pool("ps", 8, space="PSUM")
    drm = pool("drm", 2, space="DRAM")

    cnt = [0]
    def s2(dtype=f32, shape=(P, TC)):
        cnt[0] += 1
        return mega.tile(list(shape), dtype, tag="s2", name=f"s2_{cnt[0]}")

    def sc(shape=(P, TC), dtype=f32):
        cnt[0] += 1
        return scr.tile(list(shape), dtype, tag="scr", name=f"sc_{cnt[0]}")

    def pst_(shape=(P, TC), dtype=f32):
        cnt[0] += 1
        return ps.tile(list(shape), dtype, tag="ps", name=f"ps_{cnt[0]}")

    # ---------------- constants ----------------
    IOTA_T = const.tile([P, TC], f32, tag="iota_t")
    nc.gpsimd.iota(IOTA_T[:], pattern=[[1, TC]], base=0, channel_multiplier=0,
                   allow_small_or_imprecise_dtypes=True)
    IOTA_P = const.tile([P, 1], f32, tag="iota_p")
    nc.gpsimd.iota(IOTA_P[:], pattern=[[1, 1]], base=0, channel_multiplier=1,
                   allow_small_or_imprecise_dtypes=True)
    IDENT = const.tile([P, P], mmdt, tag="ident")
    make_identity(nc, IDENT[:])
    IDENT2 = const.tile([P, S], mmdt, tag="ident2")
    ONESPS = sc((P, S))
    nc.gpsimd.memset(ONESPS[:], 1.0)
    id2a = sc((P, S))
    nc.gpsimd.affine_select(id2a[:], ONESPS[:], pattern=[[1, S]], base=S,
                            channel_multiplier=-1, compare_op=AOT.is_ge, fill=0.0)
    nc.gpsimd.affine_select(IDENT2[:], id2a[:], pattern=[[-1, S]], base=-S,
                            channel_multiplier=1, compare_op=AOT.is_ge, fill=0.0)
    ONES_K = const.tile([P, 1], f32, tag="ones_k")
    nc.gpsimd.memset(ONES_K[:], 1.0)
    EPSB = const.tile([P, 1], f32, tag="epsb")
    nc.gpsimd.memset(EPSB[:], EPS)
    BLKP = const.tile([P, 2], f32, tag="blkp")
    nc.gpsimd.memset(BLKP[:], 0.0)
    nc.gpsimd.memset(BLKP[0:S, 0:1], 1.0)
    nc.gpsimd.memset(BLKP[S:P, 1:2], 1.0)
    SEL2 = const.tile([2, P], f32, tag="sel2")
    ones2 = sc((2, P))
    nc.gpsimd.memset(ones2[:], 1.0)
    sel_a = sc((2, P))
    nc.gpsimd.affine_select(sel_a[:], ones2[:], pattern=[[1, P]], base=0,
                            channel_multiplier=-S, compare_op=AOT.is_ge, fill=0.0)
    nc.gpsimd.affine_select(SEL2[:], sel_a[:], pattern=[[-1, P]], base=S - 1,
                            channel_multiplier=S, compare_op=AOT.is_ge, fill=0.0)
    E4M = []
    for j in range(4):
        e = sc()
        nc.gpsimd.iota(e[:], pattern=[[1, TC]], base=-(j * P) - 1,
                       channel_multiplier=-1, allow_small_or_imprecise_dtypes=True)
        em = const.tile([P, TC], f32, tag=f"e4m_{j}", name=f"e4m_{j}")
        nc.gpsimd.affine_select(em[:], e[:], pattern=[[1, TC]], base=-(j * P) - 1,
                                channel_multiplier=-1, compare_op=AOT.is_ge, fill=1e30)
        E4M.append(em)

    def ld(name, cols):
        t = const.tile([P, cols], f32, tag=name, name=name)
        nc.sync.dma_start(t[:], dparam(name)[:])
        return t

    LN1G = ld("ln1g", NK); LN1B = ld("ln1b", NK)
    LN2G = ld("ln2g", NK); LN2B = ld("ln2b", NK)
    MXK = ld("mxk", NK); MXV = ld("mxv", NK); MXR = ld("mxr", NK)
    FMK = ld("fmk", NK); FMR = ld("fmr", NK)
    LNXG = ld("lnxg", NP); LNXB = ld("lnxb", NP)

    def onem(src, name):
        t = const.tile([P, NK], f32, tag=name, name=name)
        nc.vector.tensor_scalar(t[:], src[:], -1.0, 1.0, AOT.mult, AOT.add)
        return t
    MXK1 = onem(MXK, "mxk1"); MXV1 = onem(MXV, "mxv1"); MXR1 = onem(MXR, "mxr1")
    FMK1 = onem(FMK, "fmk1"); FMR1 = onem(FMR, "fmr1")

    TD = const.tile([P, HL], f32, tag="td")
    nc.sync.dma_start(TD[:], dparam("tdv")[0:1, :].partition_broadcast(P))
    UU = const.tile([P, HL], f32, tag="uu")
    nc.sync.dma_start(UU[:], dparam("uv")[0:1, :].partition_broadcast(P))
    NEGLNW = const.tile([P, HL], f32, tag="neglnw")
    nc.scalar.activation(NEGLNW[:], TD[:], AFT.Exp)
    LNW = const.tile([P, HL], f32, tag="lnw")
    nc.vector.tensor_scalar_mul(LNW[:], NEGLNW[:], -1.0)
    WS = const.tile([P, HL], f32, tag="ws")
    nc.scalar.activation(WS[:], LNW[:], AFT.Exp, scale=float(TC))
    LNWB = const.tile([P, HL * 4], f32, tag="lnwb")
    WKC = const.tile([P, HL * 4], f32, tag="wkc")
    for h in range(HL):
        for j in range(4):
            idx = h * 4 + j
            nc.vector.tensor_scalar_mul(LNWB[:, idx:idx + 1], LNW[:, h:h + 1],
                                        float(TC - 1 - j * P))
            nc.scalar.activation(WKC[:, idx:idx + 1], IOTA_P[:], AFT.Exp,
                                 bias=LNWB[:, idx:idx + 1],
                                 scale=NEGLNW[:, h:h + 1])

    STATE = [pers.tile([P, S], f32, tag=f"state_{p}", name=f"state_{p}") for p in range(NP)]
    for t in STATE:
        nc.gpsimd.memset(t[:], 0.0)
    H1HALO = pers.tile([P, NK], f32, tag="h1halo", name="h1halo")
    nc.gpsimd.memset(H1HALO[:], 0.0)
    H2HALO = pers.tile([P, NK], f32, tag="h2halo", name="h2halo")
    nc.gpsimd.memset(H2HALO[:], 0.0)

    xT = dparam("xT"); yT = dparam("yT")

    # ---------- helpers ----------
    def ln_stats(src_fn):
        pss = pst_((1, TC))
        psq = pst_((1, TC))
        for k in range(NK):
            t = src_fn(k)
            sq = sc()
            nc.scalar.square(sq[:], t[:])
            nc.tensor.matmul(pss[:], ONES_K[:], t[:], start=(k == 0), stop=(k == NK - 1))
            nc.tensor.matmul(psq[:], ONES_K[:], sq[:], start=(k == 0), stop=(k == NK - 1))
        m_ = sc((1, TC)); nc.scalar.mul(m_[:], pss[:], 1.0 / C)
        q_ = sc((1, TC)); nc.scalar.mul(q_[:], psq[:], 1.0 / C)
        msq = sc((1, TC)); nc.scalar.square(msq[:], m_[:])
        var = sc((1, TC)); nc.vector.tensor_sub(var[:], q_[:], msq[:])
        sd = sc((1, TC))
        nc.scalar.activation(sd[:], var[:], AFT.Sqrt, bias=EPSB[0:1, 0:1])
        rs = sc((1, TC)); nc.vector.reciprocal(rs[:], sd[:])
        mrs = sc((1, TC))
        nc.vector.scalar_tensor_tensor(mrs[:], m_[:], -1.0, rs[:], AOT.mult, AOT.mult)
        brs = sc(); nc.gpsimd.partition_broadcast(brs[:], rs[:])
        bmrs = sc(); nc.gpsimd.partition_broadcast(bmrs[:], mrs[:])
        return brs, bmrs

    def normalize(dst, src, brs, bmrs, g, b, k):
        nc.vector.tensor_mul(dst[:], src[:], brs[:])
        nc.vector.tensor_add(dst[:], dst[:], bmrs[:])
        nc.vector.tensor_scalar(dst[:], dst[:], g[:, k:k + 1], b[:, k:k + 1],
                                AOT.mult, AOT.add)

    def mix(dst, h1t, halo_col, cf, cf1, k):
        nc.vector.tensor_scalar(dst[:], h1t[:], cf[:, k:k + 1], None, AOT.mult)
        nc.vector.scalar_tensor_tensor(dst[:, 1:TC], h1t[:, 0:TC - 1], cf1[:, k:k + 1],
                                       dst[:, 1:TC], AOT.mult, AOT.add)
        nc.vector.scalar_tensor_tensor(dst[:, 0:1], halo_col, cf1[:, k:k + 1],
                                       dst[:, 0:1], AOT.mult, AOT.add)

    # ================= chunk phases =================
    def timemix(i):
        t0 = i * TC
        def src_x(k):
            t = sc()
            nc.sync.dma_start(t[:], xT[k * P:(k + 1) * P, t0:t0 + TC])
            return t
        brs, bmrs = ln_stats(src_x)
        H1 = []
        for k in range(NK):
            t = src_x(k)
            h1 = s2()
            normalize(h1, t, brs, bmrs, LN1G, LN1B, k)
            H1.append(h1)
        # --- projections ---
        RT = [None] * NP; KT = [None] * NP; VS = [None] * 8
        for q, (wnm, CF, CF1) in enumerate(
                [("wr", MXR, MXR1), ("wk", MXK, MXK1), ("wv", MXV, MXV1)]):
            Wq = dparam(wnm)
            psq = [pst_((P, TC)) for _ in range(NP)]
            for k in range(NK):
                mq = sc(dtype=mmdt)
                mix(mq, H1[k], H1HALO[:, k:k + 1], CF, CF1, k)
                wt = wts.tile([P, CH], mmdt, tag="wt")
                nc.sync.dma_start(wt[:], Wq[k * P:(k + 1) * P, :])
                if q < 2:
                    for m in range(NP):
                        nc.tensor.matmul(psq[m][:], wt[:, m * P:(m + 1) * P], mq[:],
                                         start=(k == 0), stop=(k == NK - 1))
                else:
                    for t4 in range(4):
                        for cg in range(2):
                            nc.tensor.matmul(psq[t4 * 2 + cg][:],
                                             mq[:, t4 * P:(t4 + 1) * P],
                                             wt[:, cg * TC:(cg + 1) * TC],
                                             start=(k == 0), stop=(k == NK - 1))
            for m in range(NP):
                dst = s2(mmdt)
                nc.vector.tensor_copy(dst[:], psq[m][:])
                if q == 0:
                    RT[m] = dst
                elif q == 1:
                    KT[m] = dst
                else:
                    VS[m] = dst
        for k in range(NK):
            nc.vector.tensor_copy(H1HALO[:, k:k + 1], H1[k][:, TC - 1:TC])
        H1 = None  # released after halo copies
        # --- attention ---
        XA = [None] * NP
        for p in range(NP):
            wb = sc()  # w^t rows for head pair
            nc.scalar.activation(wb[0:S, :], IOTA_T[0:S, :], AFT.Exp,
                                 scale=LNW[0:S, 2 * p:2 * p + 1])
            nc.scalar.activation(wb[S:P, :], IOTA_T[S:P, :], AFT.Exp,
                                 scale=LNW[S:P, 2 * p + 1:2 * p + 2])
            rtw = sc(dtype=mmdt)
            pout = pst_((P, TC))
            for hh in range(2):
                h = 2 * p + hh
                pr = slice(hh * S, hh * S + S)
                nc.vector.tensor_mul(rtw[pr, :], RT[p][pr, :], wb[pr, :])
                if mmdt is f32:
                    st_mm = STATE[p]
                else:
                    st_mm = sc((P, S), mmdt)
                    nc.vector.tensor_copy(st_mm[pr, :], STATE[p][pr, :])
                nc.tensor.matmul(pout[pr, :], st_mm[pr, :], rtw[pr, :],
                                 start=True, stop=False)
                pst = pst_((P, S))
                for j in range(4):
                    wmt = sc()
                    nc.scalar.activation(wmt[:], E4M[j][:], AFT.Exp,
                                         scale=LNW[:, h:h + 1])
                    nc.vector.scalar_tensor_tensor(
                        wmt[:, j * P:(j + 1) * P], IDENT[:, 0:P], UU[:, h:h + 1],
                        wmt[:, j * P:(j + 1) * P], AOT.mult, AOT.add)
                    pa = pst_((P, TC))
                    nc.tensor.matmul(pa[:], KT[p][pr, j * P:(j + 1) * P],
                                     RT[p][pr, :], start=True, stop=True)
                    ast = sc(dtype=mmdt)
                    nc.vector.tensor_mul(ast[:], pa[:], wmt[:])
                    vsl = VS[j * 2 + (h * S) // TC]
                    voff = (h * S) % TC
                    nc.tensor.matmul(pout[pr, :], vsl[:, voff:voff + S], ast[:],
                                     start=False, stop=(j == 3))
                    ptr = pst_((P, S), mmdt)
                    ident = IDENT[0:S, 0:S] if hh == 0 else IDENT2[S:P, :]
                    nc.tensor.transpose(ptr[:], KT[p][pr, j * P:(j + 1) * P], ident)
                    kkwt = sc((P, S), mmdt)
                    nc.scalar.mul(kkwt[:], ptr[:], WKC[:, h * 4 + j:h * 4 + j + 1])
                    nc.tensor.matmul(pst[pr, :], kkwt[:], vsl[:, voff:voff + S],
                                     start=(j == 0), stop=(j == 3))
                nc.vector.scalar_tensor_tensor(STATE[p][pr, :], STATE[p][pr, :],
                                               WS[pr, h:h + 1], pst[pr, :],
                                               AOT.mult, AOT.add)
            xa = s2()
            nc.vector.tensor_copy(xa[:], pout[:])
            XA[p] = xa
        RT = KT = VS = None
        # --- groupnorm ---
        for p in range(NP):
            sq = sc()
            nc.scalar.square(sq[:], XA[p][:])
            pgs = pst_((2, TC))
            nc.tensor.matmul(pgs[:], BLKP[:], XA[p][:], start=True, stop=True)
            pgq = pst_((2, TC))
            nc.tensor.matmul(pgq[:], BLKP[:], sq[:], start=True, stop=True)
            m_ = sc((2, TC)); nc.scalar.mul(m_[:], pgs[:], 1.0 / (S * HS_DIV))
            q_ = sc((2, TC)); nc.scalar.mul(q_[:], pgq[:], 1.0 / (S * HS_DIV * HS_DIV))
            msq = sc((2, TC)); nc.scalar.square(msq[:], m_[:])
            var = sc((2, TC)); nc.vector.tensor_sub(var[:], q_[:], msq[:])
            sd = sc((2, TC))
            nc.scalar.activation(sd[:], var[:], AFT.Sqrt, bias=EPSB[0:2, 0:1])
            rs = sc((2, TC)); nc.vector.reciprocal(rs[:], sd[:])
            mrs = sc((2, TC))
            nc.vector.scalar_tensor_tensor(mrs[:], m_[:], -1.0, rs[:], AOT.mult, AOT.mult)
            rsh = sc((2, TC)); nc.vector.tensor_scalar_mul(rsh[:], rs[:], 1.0 / HS_DIV)
            pbr = pst_((P, TC))
            nc.tensor.matmul(pbr[:], SEL2[:], rsh[:], start=True, stop=True)
            pbm = pst_((P, TC))
            nc.tensor.matmul(pbm[:], SEL2[:], mrs[:], start=True, stop=True)
            nc.vector.tensor_mul(XA[p][:], XA[p][:], pbr[:])
            nc.vector.tensor_add(XA[p][:], XA[p][:], pbm[:])
            nc.vector.tensor_scalar(XA[p][:], XA[p][:], LNXG[:, p:p + 1],
                                    LNXB[:, p:p + 1], AOT.mult, AOT.add)
        if mmdt is not f32:
            XAB = []
            for p in range(NP):
                xb = s2(mmdt)
                nc.vector.tensor_copy(xb[:], XA[p][:])
                XAB.append(xb)
            XA = XAB
        # --- Wo partial -> AllReduce ---
        xo_in = drm.tile([C, TC], f32, tag="xoin")
        for g in range(2):
            psw = [pst_((P, TC)) for _ in range(NP)]
            for k in range(NP):
                wt = wts.tile([P, CH], mmdt, tag="wt")
                nc.sync.dma_start(wt[:], dparam("wo")[k * P:(k + 1) * P,
                                                      g * CH:(g + 1) * CH])
                for m in range(NP):
                    nc.tensor.matmul(psw[m][:], wt[:, m * P:(m + 1) * P], XA[k][:],
                                     start=(k == 0), stop=(k == NP - 1))
            for m in range(NP):
                ev = sc()
                nc.vector.tensor_copy(ev[:], psw[m][:])
                nc.sync.dma_start(xo_in[(g * NP + m) * P:(g * NP + m + 1) * P, :], ev[:])
        xo_out = drm.tile([C, TC], f32, tag="xoout")
        nc.gpsimd.collective_compute("AllReduce", AOT.add, replica_groups=GROUPS,
                                     ins=[xo_in.opt()], outs=[xo_out.opt()])
        return xo_out

    def ffn(i, xo_out):
        t0 = i * TC
        def src_x2(k):
            ta = sc()
            nc.sync.dma_start(ta[:], xo_out[k * P:(k + 1) * P, :])
            tx = sc()
            nc.sync.dma_start(tx[:], xT[k * P:(k + 1) * P, t0:t0 + TC])
            nc.vector.tensor_add(ta[:], ta[:], tx[:])
            return ta
        brs, bmrs = ln_stats(src_x2)
        MFK = []; MFR = []
        for k in range(NK):
            t = src_x2(k)
            h2 = sc()
            normalize(h2, t, brs, bmrs, LN2G, LN2B, k)
            mfr = s2(bf16); mix(mfr, h2, H2HALO[:, k:k + 1], FMR, FMR1, k)
            mfk = s2(bf16); mix(mfk, h2, H2HALO[:, k:k + 1], FMK, FMK1, k)
            MFR.append(mfr); MFK.append(mfk)
            nc.vector.tensor_copy(H2HALO[:, k:k + 1], h2[:, TC - 1:TC])
        # --- Wfk -> relu^2 ---
        KF = []
        for g in range(4):
            psf = [pst_((P, TC)) for _ in range(7)]
            for k in range(NK):
                wt = wts.tile([P, 7 * P], bf16, tag="wt")
                nc.sync.dma_start(wt[:], dparam("wfk")[k * P:(k + 1) * P,
                                                       g * 7 * P:(g + 1) * 7 * P])
                for m in range(7):
                    nc.tensor.matmul(psf[m][:], wt[:, m * P:(m + 1) * P], MFK[k][:],
                                     start=(k == 0), stop=(k == NK - 1))
            for m in range(7):
                rl = sc()
                nc.scalar.activation(rl[:], psf[m][:], AFT.Relu)
                kf = s2(bf16)
                nc.vector.tensor_mul(kf[:], rl[:], rl[:])
                KF.append(kf)
        MFK = None
        # --- Wfv -> kv partial -> AllReduce ---
        kv_in = drm.tile([C, TC], f32, tag="kvin")
        for g in range(2):
            psv = [pst_((P, TC)) for _ in range(NP)]
            for j in range(NDF):
                wt = wts.tile([P, CH], bf16, tag="wt")
                nc.sync.dma_start(wt[:], dparam("wfv")[j * P:(j + 1) * P,
                                                       g * CH:(g + 1) * CH])
                for m in range(NP):
                    nc.tensor.matmul(psv[m][:], wt[:, m * P:(m + 1) * P], KF[j][:],
                                     start=(j == 0), stop=(j == NDF - 1))
            for m in range(NP):
                ev = sc()
                nc.vector.tensor_copy(ev[:], psv[m][:])
                nc.sync.dma_start(kv_in[(g * NP + m) * P:(g * NP + m + 1) * P, :], ev[:])
        KF = None
        kv_out = drm.tile([C, TC], f32, tag="kvout")
        nc.gpsimd.collective_compute("AllReduce", AOT.add, replica_groups=GROUPS,
                                     ins=[kv_in.opt()], outs=[kv_out.opt()])
        # --- Wfr -> sigmoid (overlaps AllReduce) ---
        GT = []
        for g in range(2):
            psg = [pst_((P, TC)) for _ in range(NP)]
            for k in range(NK):
                wt = wts.tile([P, CH], bf16, tag="wt")
                nc.sync.dma_start(wt[:], dparam("wfr")[k * P:(k + 1) * P,
                                                       g * CH:(g + 1) * CH])
                for m in range(NP):
                    nc.tensor.matmul(psg[m][:], wt[:, m * P:(m + 1) * P], MFR[k][:],
                                     start=(k == 0), stop=(k == NK - 1))
            for m in range(NP):
                gt = s2(bf16)
                nc.scalar.activation(gt[:], psg[m][:], AFT.Sigmoid)
                GT.append(gt)
        MFR = None
        # --- final combine ---
        for m in range(NK):
            tkv = sc()
            nc.sync.dma_start(tkv[:], kv_out[m * P:(m + 1) * P, :])
            nc.vector.tensor_mul(tkv[:], tkv[:], GT[m][:])
            ta = sc()
            nc.sync.dma_start(ta[:], xo_out[m * P:(m + 1) * P, :])
            tx = sc()
            nc.sync.dma_start(tx[:], xT[m * P:(m + 1) * P, t0:t0 + TC])
            nc.vector.tensor_add(ta[:], ta[:], tx[:])
            nc.vector.tensor_add(ta[:], ta[:], tkv[:])
            nc.sync.dma_start(yT[m * P:(m + 1) * P, t0:t0 + TC], ta[:])

    xo_prev = None
    for i in range(NCH):
        xo = timemix(i)
        if xo_prev is not None:
            ffn(i - 1, xo_prev)
        xo_prev = xo
    ffn(NCH - 1, xo_prev)

    for c in reversed(ctxs):
        c.__exit__(None, None, None)


# ----------------------------------------------------------------------
# Host-side sharding / gather
# ----------------------------------------------------------------------
import ml_dtypes

_NC_CACHE = {}


def _vec_pk(v, nk=NK):
    return np.ascontiguousarray(np.asarray(v).reshape(nk, P).T.astype(np.float32))


def _make_in_maps(inputs, T, mmdt_key):
    x = np.asarray(inputs["x"], np.float32)
    bf = ml_dtypes.bfloat16
    mmnp = np.float32 if mmdt_key == "f32" else bf
    maps = []
    for c in range(8):
        b, hh = c // 2, c % 2
        sl = slice(hh * CH, (hh + 1) * CH)
        dsl = slice(hh * DFH, (hh + 1) * DFH)
        hsl = slice(hh * HL, (hh + 1) * HL)
        maps.append({
            "xT": np.ascontiguousarray(x[b, :T, :].T),
            "wr": np.ascontiguousarray(np.asarray(inputs["Wr"], np.float32)[:, sl].astype(mmnp)),
            "wk": np.ascontiguousarray(np.asarray(inputs["Wk"], np.float32)[:, sl].astype(mmnp)),
            "wv": np.ascontiguousarray(np.asarray(inputs["Wv"], np.float32)[:, sl].astype(mmnp)),
            "wo": np.ascontiguousarray(np.asarray(inputs["Wo"], np.float32)[sl, :].astype(mmnp)),
            "wfk": np.ascontiguousarray(np.asarray(inputs["Wfk"], np.float32)[:, dsl].astype(bf)),
            "wfv": np.ascontiguousarray(np.asarray(inputs["Wfv"], np.float32)[dsl, :].astype(bf)),
            "wfr": np.ascontiguousarray(np.asarray(inputs["Wfr"], np.float32).astype(bf)),
            "ln1g": _vec_pk(inputs["ln1_g"]), "ln1b": _vec_pk(inputs["ln1_b"]),
            "ln2g": _vec_pk(inputs["ln2_g"]), "ln2b": _vec_pk(inputs["ln2_b"]),
            "mxk": _vec_pk(inputs["att_mix_k"]), "mxv": _vec_pk(inputs["att_mix_v"]),
            "mxr": _vec_pk(inputs["att_mix_r"]),
            "fmk": _vec_pk(inputs["ffn_mix_k"]), "fmr": _vec_pk(inputs["ffn_mix_r"]),
            "lnxg": _vec_pk(np.asarray(inputs["lnx_g"])[sl], NP),
            "lnxb": _vec_pk(np.asarray(inputs["lnx_b"])[sl], NP),
            "tdv": np.ascontiguousarray(np.asarray(inputs["time_decay"], np.float32)[hsl][None, :]),
            "uv": np.ascontiguousarray(np.asarray(inputs["time_faaaa"], np.float32)[hsl][None, :]),
        })
    return maps


def run_on_hw(inputs, T=2048, mmdt_key="bf16", trace=False):
    from concourse.bass_utils import run_bass_kernel_spmd
    key = (T, mmdt_key)
    if key not in _NC_CACHE:
        _NC_CACHE[key] = build_nc(T=T, mmdt=bf16 if mmdt_key == "bf16" else f32)
    nc = _NC_CACHE[key]
    maps = _make_in_maps(inputs, T, mmdt_key)
    res = run_bass_kernel_spmd(nc, maps, core_ids=list(range(8)), trace=trace)
    B = 4
    out = np.zeros((B, T, C), np.float32)
    for b in range(B):
        out[b] = res.results[2 * b]["yT"].T
    return out, res


def kernel(**inputs) -> np.ndarray:
    out, _ = run_on_hw(inputs, T=2048, mmdt_key="bf16", trace=False)
    return out



# revision 2
# speedup vs baseline: 1.0081x; 1.0081x over previous
"""RWKV5 block, sequence-parallel across 8 trn2 cores.

Core c -> batch c//2, sequence half c%2 (tokens t0 = half*1024, TL=1024
= 2 recurrence chunks of TC=512). Each core runs FULL-width GEMMs
(C=2048, DF=7168) on its token half; every weight is streamed from HBM
once (Wv twice). Cross-core traffic per pair: one 512KB state AllGather
(recurrent state after chunk 1 -> second half) plus an 8KB x' halo
column AllGather for the ChannelMix time-shift.

Layout: activations channel-major [C, T]. v kept time-major [T, C]
(VT) for the attention a@v and k^T@v contractions.
"""
import numpy as np
import concourse.bass as bass
import concourse.mybir as mybir
import concourse.tile as tile
from concourse import bacc
from concourse.masks import make_identity

f32 = mybir.dt.float32
bf16 = mybir.dt.bfloat16
AOT = mybir.AluOpType
AFT = mybir.ActivationFunctionType

C = 2048
H = 32         # heads
S = 64         # head dim
TC = 512       # recurrence chunk
TL = 1024      # local tokens per core
NCH = TL // TC # 2 local chunks
DF = 7168
P = 128
NK = C // P    # 16 channel chunks
NP = H // 2    # 16 head pairs
NJ = DF // P   # 56
NQ = 4         # DF quarters
JQ = NJ // NQ  # 14 j-chunks per quarter
EPS = 1e-5
HS_DIV = float(np.sqrt(S))
GROUPS = [[0, 1], [2, 3], [4, 5], [6, 7]]
TS = TL // TC  # 2 column sub-ranges of 512


def build_nc():
    nc = bacc.Bacc("TRN2", target_bir_lowering=False, debug=False, num_devices=8)
    dp = nc.declare_dram_parameter
    params = {
        "xT": dp("xT", [C, 1 + TL], f32, isOutput=False),
        # weights pre-tiled on host: cols ordered (m-group, k, col-in-tile)
        "wr_t": dp("wr_t", [P, C * C // P], bf16, isOutput=False),
        "wk_t": dp("wk_t", [P, C * C // P], bf16, isOutput=False),
        "wv_t": dp("wv_t", [P, C * C // P], bf16, isOutput=False),
        "wo_t": dp("wo_t", [P, C * C // P], bf16, isOutput=False),
        "wfk_t": dp("wfk_t", [P, C * DF // P], bf16, isOutput=False),
        "wfv_t": dp("wfv_t", [P, C * DF // P], bf16, isOutput=False),
        "wfr_t": dp("wfr_t", [P, C * C // P], bf16, isOutput=False),
        "wkcpp": dp("wkcpp", [P, H * 4], f32, isOutput=False),
        "wspp": dp("wspp", [P, NP], f32, isOutput=False),
        "smask": dp("smask", [1, 1], f32, isOutput=False),
        "tdv": dp("tdv", [1, H], f32, isOutput=False),
        "uv": dp("uv", [1, H], f32, isOutput=False),
        "yT": dp("yT", [C, TL], f32, isOutput=True),
    }
    for nm, cols in [("ln1g", NK), ("ln1b", NK), ("ln2g", NK), ("ln2b", NK),
                     ("mxk", NK), ("mxv", NK), ("mxr", NK), ("fmk", NK),
                     ("fmr", NK), ("lnxg", NP), ("lnxb", NP)]:
        params[nm] = dp(nm, [P, cols], f32, isOutput=False)
    with tile.TileContext(nc) as tc:
        _build(nc, tc, params)
    nc.compile()
    return nc


def _build(nc, tc, params):
    ctxs = []

    def pool(name, bufs, space="SBUF"):
        p = tc.tile_pool(name=name, bufs=bufs, space=space)
        ctxs.append(p)
        return p.__enter__()

    const = pool("const", 1)
    pers = pool("pers", 1)
    big = pool("big", 65)          # [P,1+TL]-bf16-slab activation tiles
    scr = pool("scr", 8)           # [P,TC]-f32 scratch
    xsrc = pool("xsrc", 2)         # [P,1+TL]-f32 streamed sources
    sscr = pool("sscr", 10)        # small [P,S] scratch
    wmtb = pool("wmtb", 5)         # [P,2TC]-bf16 cached two-head decay masks
    wts = pool("wts", 3)           # [128,2048]bf16 weight-blob ring
    psa = pool("psa", 3, space="PSUM")   # [P,2TC] f32 (2 banks)
    psb = pool("psb", 2, space="PSUM")   # [P,TC] f32 (1 bank)
    drm = pool("drm", 1, space="DRAM")

    cnt = [0]

    def bigt(dtype=bf16, cols=1 + TL):
        cnt[0] += 1
        return big.tile([P, cols], dtype, tag="big", name=f"b_{cnt[0]}")

    def sc(shape=(P, TC), dtype=f32):
        cnt[0] += 1
        return scr.tile(list(shape), dtype, tag="scr", name=f"sc_{cnt[0]}")

    def xsc():
        cnt[0] += 1
        return xsrc.tile([P, 1 + TL], f32, tag="xsrc", name=f"xs_{cnt[0]}")

    def ssc(shape=(P, S), dtype=f32):
        cnt[0] += 1
        return sscr.tile(list(shape), dtype, tag="sscr", name=f"ss_{cnt[0]}")

    def wmt_tile():
        cnt[0] += 1
        return wmtb.tile([P, 2 * TC], bf16, tag="wmtb", name=f"wm_{cnt[0]}")

    def wt_tile(cols=2048):
        cnt[0] += 1
        return wts.tile([P, cols], bf16, tag="wt", name=f"wt_{cnt[0]}")

    def psa_():
        cnt[0] += 1
        return psa.tile([P, 2 * TC], f32, tag="psa", name=f"pa_{cnt[0]}")

    def pst_(shape=(P, TC), dtype=f32):
        cnt[0] += 1
        return psb.tile(list(shape), dtype, tag="psb", name=f"pb_{cnt[0]}")

    # ---------------- constants ----------------
    IOTA_T = const.tile([P, TC], f32, tag="iota_t")
    nc.gpsimd.iota(IOTA_T[:], pattern=[[1, TC]], base=0, channel_multiplier=0,
                   allow_small_or_imprecise_dtypes=True)
    IDENT = const.tile([P, P], bf16, tag="ident")
    make_identity(nc, IDENT[:])
    IDENT2 = const.tile([P, S], bf16, tag="ident2")
    ONESPS = sc((P, S))
    nc.gpsimd.memset(ONESPS[:], 1.0)
    id2a = sc((P, S))
    nc.gpsimd.affine_select(id2a[:], ONESPS[:], pattern=[[1, S]], base=S,
                            channel_multiplier=-1, compare_op=AOT.is_ge, fill=0.0)
    nc.gpsimd.affine_select(IDENT2[:], id2a[:], pattern=[[-1, S]], base=-S,
                            channel_multiplier=1, compare_op=AOT.is_ge, fill=0.0)
    ONES_K = const.tile([P, 1], f32, tag="ones_k")
    nc.gpsimd.memset(ONES_K[:], 1.0)
    ONES_KB = const.tile([P, 1], bf16, tag="ones_kb")
    nc.gpsimd.memset(ONES_KB[:], 1.0)
    BLKPB = const.tile([P, 2], bf16, tag="blkpb")
    nc.gpsimd.memset(BLKPB[:], 0.0)
    nc.gpsimd.memset(BLKPB[0:S, 0:1], 1.0)
    nc.gpsimd.memset(BLKPB[S:P, 1:2], 1.0)
    # IOTAW[p, j*64+c] = 511 - 128*j - p  (contrib decay exponents)
    IOTAW = const.tile([P, 4 * S], f32, tag="iotaw")
    nc.gpsimd.iota(IOTAW[:], pattern=[[-P, 4], [0, S]], base=TC - 1,
                   channel_multiplier=-1, allow_small_or_imprecise_dtypes=True)
    EPSB = const.tile([P, 1], f32, tag="epsb")
    nc.gpsimd.memset(EPSB[:], EPS)
    BLKP = const.tile([P, 2], f32, tag="blkp")
    nc.gpsimd.memset(BLKP[:], 0.0)
    nc.gpsimd.memset(BLKP[0:S, 0:1], 1.0)
    nc.gpsimd.memset(BLKP[S:P, 1:2], 1.0)
    SEL2 = const.tile([2, P], f32, tag="sel2")
    ones2 = sc((2, P))
    nc.gpsimd.memset(ones2[:], 1.0)
    sel_a = sc((2, P))
    nc.gpsimd.affine_select(sel_a[:], ones2[:], pattern=[[1, P]], base=0,
                            channel_multiplier=-S, compare_op=AOT.is_ge, fill=0.0)
    nc.gpsimd.affine_select(SEL2[:], sel_a[:], pattern=[[-1, P]], base=S - 1,
                            channel_multiplier=S, compare_op=AOT.is_ge, fill=0.0)
    E4M = []
    for j in range(4):
        e = sc()
        nc.gpsimd.iota(e[:], pattern=[[1, TC]], base=-(j * P) - 1,
                       channel_multiplier=-1, allow_small_or_imprecise_dtypes=True)
        em = const.tile([P, TC], f32, tag=f"e4m_{j}", name=f"e4m_{j}")
        nc.gpsimd.affine_select(em[:], e[:], pattern=[[1, TC]], base=-(j * P) - 1,
                                channel_multiplier=-1, compare_op=AOT.is_ge, fill=1e30)
        E4M.append(em)

    def ld(name, cols):
        t = const.tile([P, cols], f32, tag=name, name=name)
        nc.sync.dma_start(t[:], params[name][:])
        return t

    LN1G = ld("ln1g", NK); LN1B = ld("ln1b", NK)
    LN2G = ld("ln2g", NK); LN2B = ld("ln2b", NK)
    MXK = ld("mxk", NK); MXV = ld("mxv", NK); MXR = ld("mxr", NK)
    FMK = ld("fmk", NK); FMR = ld("fmr", NK)
    LNXG = ld("lnxg", NP); LNXB = ld("lnxb", NP)
    WKC = ld("wkcpp", H * 4)
    WSPP = ld("wspp", NP)

    def onem(src, name):
        t = const.tile([P, NK], f32, tag=name, name=name)
        nc.vector.tensor_scalar(t[:], src[:], -1.0, 1.0, AOT.mult, AOT.add)
        return t
    MXK1 = onem(MXK, "mxk1"); MXV1 = onem(MXV, "mxv1"); MXR1 = onem(MXR, "mxr1")
    FMK1 = onem(FMK, "fmk1"); FMR1 = onem(FMR, "fmr1")

    TD = const.tile([P, H], f32, tag="td")
    nc.sync.dma_start(TD[:], params["tdv"][0:1, :].partition_broadcast(P))
    UU = const.tile([P, H], f32, tag="uu")
    nc.sync.dma_start(UU[:], params["uv"][0:1, :].partition_broadcast(P))
    SMB = const.tile([P, 1], f32, tag="smb")
    nc.sync.dma_start(SMB[:], params["smask"][0:1, :].partition_broadcast(P))
    NEGLNW = const.tile([P, H], f32, tag="neglnw")
    nc.scalar.activation(NEGLNW[:], TD[:], AFT.Exp)
    LNW = const.tile([P, H], f32, tag="lnw")
    nc.vector.tensor_scalar_mul(LNW[:], NEGLNW[:], -1.0)

    xT = params["xT"]; yT = params["yT"]

    # DRAM tiles: collectives + x' spill
    sout_d = drm.tile([P, NP * S], f32, tag="soutd")
    sgat_d = drm.tile([2 * P, NP * S], f32, tag="sgatd")
    xcol_d = drm.tile([P, NK], f32, tag="xcold")
    xcgat_d = drm.tile([2 * P, NK], f32, tag="xcgatd")
    xprime_d = drm.tile([C, TL], bf16, tag="xprd")

    # column sub-ranges of the local [*, 1+TL] tensors: halo + 2x512
    RANGES = [(0, 1)] + [(1 + i * TC, TC) for i in range(TS)]

    # ---------- layernorm over channel dim for a streamed source ----------
    def ln_pass(src_fn, g, b, dst_tiles, halo_mask):
        """src_fn(k) -> [P, 1+TL] f32 tile (fresh each call; called twice
        per k). Writes normalized bf16 into dst_tiles[k] ([P, 1+TL])."""
        stats = []  # per range: (brs, bmrs) broadcast tiles
        pssA = psa_()   # rows 0:1; cols ts*TC per main range
        psqA = psa_()
        pssh = pst_((1, 1)); psqh = pst_((1, 1))
        psr = [(pssh[:], psqh[:])] + \
              [(pssA[0:1, i * TC:(i + 1) * TC], psqA[0:1, i * TC:(i + 1) * TC])
               for i in range(TS)]
        for k in range(NK):
            t = src_fn(k)
            for ri, (off, ln) in enumerate(RANGES):
                pss, psq = psr[ri]
                sq = sc((P, ln))
                nc.vector.tensor_mul(sq[:], t[:, off:off + ln], t[:, off:off + ln])
                nc.tensor.matmul(pss, ONES_K[:], t[:, off:off + ln],
                                 start=(k == 0), stop=(k == NK - 1))
                nc.tensor.matmul(psq, ONES_K[:], sq[:],
                                 start=(k == 0), stop=(k == NK - 1))
        for ri, (off, ln) in enumerate(RANGES):
            pss, psq = psr[ri]
            m_ = sc((1, ln)); nc.scalar.mul(m_[:], pss, 1.0 / C)
            q_ = sc((1, ln)); nc.scalar.mul(q_[:], psq, 1.0 / C)
            msq = sc((1, ln)); nc.scalar.square(msq[:], m_[:])
            var = sc((1, ln)); nc.vector.tensor_sub(var[:], q_[:], msq[:])
            lnv = sc((1, ln))
            nc.scalar.activation(lnv[:], var[:], AFT.Ln, bias=EPSB[0:1, 0:1])
            rs = sc((1, ln))
            nc.scalar.activation(rs[:], lnv[:], AFT.Exp, scale=-0.5)
            mrs = sc((1, ln))
            nc.vector.scalar_tensor_tensor(mrs[:], m_[:], -1.0, rs[:],
                                           AOT.mult, AOT.mult)
            brs = sc((P, ln)); nc.gpsimd.partition_broadcast(brs[:], rs[:])
            bmrs = sc((P, ln)); nc.gpsimd.partition_broadcast(bmrs[:], mrs[:])
            stats.append((brs, bmrs))
        for k in range(NK):
            t = src_fn(k)
            dst = dst_tiles[k]
            for ri, (off, ln) in enumerate(RANGES):
                brs, bmrs = stats[ri]
                tmp = sc((P, ln))
                nc.vector.tensor_mul(tmp[:], t[:, off:off + ln], brs[:])
                nc.vector.tensor_add(tmp[:], tmp[:], bmrs[:])
                nc.vector.tensor_scalar(dst[:, off:off + ln], tmp[:],
                                        g[:, k:k + 1], b[:, k:k + 1],
                                        AOT.mult, AOT.add)
            if halo_mask:
                nc.vector.tensor_scalar(dst[:, 0:1], dst[:, 0:1],
                                        SMB[:, 0:1], None, AOT.mult)

    def mix(dst, h, cf, cf1, k):
        """dst[:, 0:TL] = cf[k]*h[:, 1:1+TL] + cf1[k]*h[:, 0:TL]"""
        nc.vector.tensor_scalar(dst[:, 0:TL], h[:, 1:1 + TL], cf[:, k:k + 1],
                                None, AOT.mult)
        nc.vector.scalar_tensor_tensor(dst[:, 0:TL], h[:, 0:TL], cf1[:, k:k + 1],
                                       dst[:, 0:TL], AOT.mult, AOT.add)

    # ---------- GEMM helper: out[m] = sum_k w_tiled[.,m,k] ^T @ in[k] ----------
    def gemm_std(wt_dram, in_tiles, out_tiles, n_out, act=None, accum=False,
                 n_in=NK, G=8, col_base=0):
        """Host-pretiled weights: blob cols ordered (mg, k, 256). m-groups
        of 2; per group ceil(n_in/G) blob DMAs of [128, G*256]; two 2-bank
        PSUM tiles (one per mi), double-buffered across groups.
        act: None -> copy; 'sigmoid'; 'sqrelu'. accum: add into out."""
        nh = (n_in + G - 1) // G
        for mgl in range(n_out // 2):
            base = col_base + mgl * n_in * 2 * P
            wtl = []
            for hf in range(nh):
                kn = min(G, n_in - hf * G)
                w = wt_tile(kn * 2 * P)
                nc.sync.dma_start(
                    w[:], wt_dram[:, base + hf * G * 2 * P:
                                  base + (hf * G + kn) * 2 * P])
                wtl.append(w)
            pq = [psa_() for _ in range(2)]
            for k in range(n_in):
                w = wtl[k // G]
                co = (k % G) * 2 * P
                for mi in range(2):
                    for ts in range(TS):
                        nc.tensor.matmul(
                            pq[mi][:, ts * TC:(ts + 1) * TC],
                            w[:, co + mi * P:co + (mi + 1) * P],
                            in_tiles[k][:, ts * TC:(ts + 1) * TC],
                            start=(k == 0), stop=(k == n_in - 1))
            for mi in range(2):
                m = mgl * 2 + mi
                dst = out_tiles[m][:, 0:TL]
                src = pq[mi][0:P, 0:TL]
                if act == "sigmoid":
                    nc.scalar.activation(dst, src, AFT.Sigmoid)
                elif act == "sqrelu":
                    rl = xsc()
                    nc.scalar.activation(rl[:, 0:TL], src, AFT.Relu)
                    nc.vector.tensor_mul(dst, rl[:, 0:TL], rl[:, 0:TL])
                elif accum:
                    nc.vector.tensor_add(dst, dst, src)
                else:
                    nc.vector.tensor_copy(dst, src)

    # ================= phase 1: LN1 =================
    def src_x(k):
        t = xsc()
        nc.sync.dma_start(t[:], xT[k * P:(k + 1) * P, :])
        return t

    H1 = [bigt() for _ in range(NK)]
    ln_pass(src_x, LN1G, LN1B, H1, halo_mask=True)

    # ================= phase 2: K / V projections =================
    MQ = [bigt(cols=TL) for _ in range(NK)]
    for k in range(NK):
        mix(MQ[k], H1[k], MXK, MXK1, k)
    KT = [bigt(cols=TL) for _ in range(NK)]
    gemm_std(params["wk_t"], MQ, KT, NK)
    for k in range(NK):
        mix(MQ[k], H1[k], MXV, MXV1, k)
    # V transposed: VT[tslab (8 x 128 tokens)] as 2 tiles of [P, 1024] each.
    # wv_t blob cols ordered (cb, k, 512); stationary = MQ slab, moving = w.
    VT = [[bigt(cols=TL) for _ in range(2)] for _ in range(2 * 4)]
    for tg in range(2):       # t-slab groups of 4
        for cb in range(4):   # c_out banks of 512
            pv = [psa_() for _ in range(2)]
            for kq in range(4):
                w = wt_tile()
                nc.sync.dma_start(
                    w[:], params["wv_t"][:, (cb * NK + kq * 4) * TC:
                                         (cb * NK + kq * 4 + 4) * TC])
                for kk in range(4):
                    k = kq * 4 + kk
                    for ti in range(4):
                        tslab = tg * 4 + ti
                        nc.tensor.matmul(
                            pv[ti // 2][:, (ti % 2) * TC:(ti % 2 + 1) * TC],
                            MQ[k][:, tslab * P:(tslab + 1) * P],
                            w[:, kk * TC:(kk + 1) * TC],
                            start=(k == 0), stop=(k == NK - 1))
            for ti in range(4):
                nc.vector.tensor_copy(
                    VT[tg * 4 + ti][cb // 2][:, (cb % 2) * TC:(cb % 2 + 1) * TC],
                    pv[ti // 2][:, (ti % 2) * TC:(ti % 2 + 1) * TC])

    def vsl(i, j, h):
        """[P,S] value slice: chunk i, 128-token slab j, head h."""
        voff = h * S
        return VT[i * 4 + j][voff // TL][:, voff % TL:voff % TL + S]

    # ============ phase 3: state contributions + AllGather ============
    CONTRIB0 = [pers.tile([P, S], f32, tag=f"c0_{p}", name=f"c0_{p}")
                for p in range(NP)]
    for p in range(NP):
        wkct = []
        for hh in range(2):
            h = 2 * p + hh
            t = sc((P, 4 * S), bf16)
            nc.scalar.activation(t[:], IOTAW[:], AFT.Exp,
                                 scale=LNW[:, h:h + 1])
            wkct.append(t)
        cts = []
        for i in range(NCH):
            pst = pst_((P, S))
            for hh in range(2):
                h = 2 * p + hh
                pr = slice(hh * S, hh * S + S)
                ident = IDENT[0:S, 0:S] if hh == 0 else IDENT2[S:P, :]
                ptr4 = pst_((P, 4 * S), bf16)
                for j in range(4):
                    nc.tensor.transpose(
                        ptr4[:, j * S:(j + 1) * S],
                        KT[p][pr, i * TC + j * P:i * TC + (j + 1) * P],
                        ident)
                kkwt = sc((P, 4 * S), bf16)
                nc.vector.tensor_mul(kkwt[:], ptr4[:], wkct[hh][:])
                for j in range(4):
                    nc.tensor.matmul(pst[pr, :], kkwt[:, j * S:(j + 1) * S],
                                     vsl(i, j, h),
                                     start=(j == 0), stop=(j == 3))
            if i == 0:
                nc.vector.tensor_copy(CONTRIB0[p][:], pst[:])
                cts.append(CONTRIB0[p])
            else:
                c1 = ssc()
                nc.vector.tensor_copy(c1[:], pst[:])
                cts.append(c1)
        so = ssc()
        nc.vector.scalar_tensor_tensor(so[:], cts[0][:], WSPP[:, p:p + 1],
                                       cts[1][:], AOT.mult, AOT.add)
        nc.sync.dma_start(sout_d[:, p * S:(p + 1) * S], so[:])
    nc.gpsimd.collective_compute("AllGather", AOT.bypass, replica_groups=GROUPS,
                                 ins=[sout_d.opt()], outs=[sgat_d.opt()])

    # ================= phase 4: R projection (overlaps AG) =================
    for k in range(NK):
        mix(MQ[k], H1[k], MXR, MXR1, k)
    H1 = None
    RT = [bigt(cols=TL) for _ in range(NK)]
    gemm_std(params["wr_t"], MQ, RT, NK)
    MQ = None

    # incoming state = smask * (rank0 shard of gather)
    SIN = pers.tile([P, NP * S], f32, tag="sin", name="sin")
    nc.sync.dma_start(SIN[:], sgat_d[0:P, :])
    nc.vector.tensor_scalar(SIN[:], SIN[:], SMB[:, 0:1], None, AOT.mult)

    # ================= phase 5: attention + groupnorm =================
    # Pair-outer loop: decay masks (wmt) and wb computed once per pair and
    # reused for both chunks. Groupnorm stats gathered into [64, TC] tiles
    # (rows 2p:2p+2 = sums, rows 32+2p = sum-of-squares) and normalized in
    # one batched pass per chunk.
    XA = [bigt(cols=TL) for _ in range(NK)]
    for p in range(NP):
        wb = sc()
        nc.scalar.activation(wb[0:S, :], IOTA_T[0:S, :], AFT.Exp,
                             scale=LNW[0:S, 2 * p:2 * p + 1])
        nc.scalar.activation(wb[S:P, :], IOTA_T[S:P, :], AFT.Exp,
                             scale=LNW[S:P, 2 * p + 1:2 * p + 2])
        WMT = []
        for j in range(4):
            wmt = wmt_tile()   # cols 0:TC = head 2p, TC:2TC = head 2p+1
            for hh in range(2):
                h = 2 * p + hh
                nc.scalar.activation(wmt[:, hh * TC:(hh + 1) * TC], E4M[j][:],
                                     AFT.Exp, scale=LNW[:, h:h + 1])
                nc.vector.scalar_tensor_tensor(
                    wmt[:, hh * TC + j * P:hh * TC + (j + 1) * P],
                    IDENT[:, 0:P], UU[:, h:h + 1],
                    wmt[:, hh * TC + j * P:hh * TC + (j + 1) * P],
                    AOT.mult, AOT.add)
            WMT.append(wmt)
        for i in range(NCH):
            st_mm = ssc((P, S), bf16)
            if i == 0:
                nc.vector.tensor_copy(st_mm[:], SIN[:, p * S:(p + 1) * S])
            else:
                stt = ssc()
                nc.vector.scalar_tensor_tensor(stt[:], SIN[:, p * S:(p + 1) * S],
                                               WSPP[:, p:p + 1], CONTRIB0[p][:],
                                               AOT.mult, AOT.add)
                nc.vector.tensor_copy(st_mm[:], stt[:])
            rtw = sc(dtype=bf16)
            nc.vector.tensor_mul(rtw[:], RT[p][:, i * TC:(i + 1) * TC], wb[:])
            pout = pst_((P, TC))
            for hh in range(2):
                pr = slice(hh * S, hh * S + S)
                nc.tensor.matmul(pout[pr, :], st_mm[pr, :], rtw[pr, :],
                                 start=True, stop=False)
            for j in range(4):
                pa2 = psa_()
                for hh in range(2):
                    pr = slice(hh * S, hh * S + S)
                    nc.tensor.matmul(
                        pa2[:, hh * TC:(hh + 1) * TC],
                        KT[p][pr, i * TC + j * P:i * TC + (j + 1) * P],
                        RT[p][pr, i * TC:(i + 1) * TC],
                        start=True, stop=True)
                ast = sc((P, 2 * TC), bf16)
                nc.vector.tensor_mul(ast[:], pa2[:], WMT[j][:])
                for hh in range(2):
                    h = 2 * p + hh
                    pr = slice(hh * S, hh * S + S)
                    nc.tensor.matmul(pout[pr, :], vsl(i, j, h),
                                     ast[:, hh * TC:(hh + 1) * TC],
                                     start=False, stop=(j == 3))
            # groupnorm (per pair, Rsqrt path); XA gets normalized bf16
            nc.vector.tensor_copy(XA[p][:, i * TC:(i + 1) * TC], pout[:])
            sq = sc()
            nc.vector.tensor_mul(sq[:], XA[p][:, i * TC:(i + 1) * TC],
                                 XA[p][:, i * TC:(i + 1) * TC])
            pgs = pst_((2, TC))
            nc.tensor.matmul(pgs[:], BLKPB[:], XA[p][:, i * TC:(i + 1) * TC],
                             start=True, stop=True)
            pgq = pst_((2, TC))
            nc.tensor.matmul(pgq[:], BLKP[:], sq[:], start=True, stop=True)
            m_ = sc((2, TC)); nc.scalar.mul(m_[:], pgs[:], 1.0 / (S * HS_DIV))
            q_ = sc((2, TC)); nc.scalar.mul(q_[:], pgq[:], 1.0 / (S * HS_DIV * HS_DIV))
            msq = sc((2, TC)); nc.vector.tensor_mul(msq[:], m_[:], m_[:])
            var = sc((2, TC)); nc.vector.tensor_sub(var[:], q_[:], msq[:])
            lnv = sc((2, TC))
            nc.scalar.activation(lnv[:], var[:], AFT.Ln, bias=EPSB[0:2, 0:1])
            rs = sc((2, TC))
            nc.scalar.activation(rs[:], lnv[:], AFT.Exp, scale=-0.5)
            mrs = sc((2, TC))
            nc.vector.scalar_tensor_tensor(mrs[:], m_[:], -1.0, rs[:],
                                           AOT.mult, AOT.mult)
            rsh = sc((2, TC)); nc.vector.tensor_scalar_mul(rsh[:], rs[:], 1.0 / HS_DIV)
            pbr = pst_((P, TC))
            nc.tensor.matmul(pbr[:], SEL2[:], rsh[:], start=True, stop=True)
            pbm = pst_((P, TC))
            nc.tensor.matmul(pbm[:], SEL2[:], mrs[:], start=True, stop=True)
            xa = sc()
            nc.vector.tensor_mul(xa[:], XA[p][:, i * TC:(i + 1) * TC], pbr[:])
            nc.vector.tensor_add(xa[:], xa[:], pbm[:])
            nc.vector.tensor_scalar(XA[p][:, i * TC:(i + 1) * TC], xa[:],
                                    LNXG[:, p:p + 1], LNXB[:, p:p + 1],
                                    AOT.mult, AOT.add)
    RT = KT = VT = None

    # ================= phase 6: Wo + residual, spill x' =================
    XP = [bigt(cols=TL) for _ in range(NK)]
    gemm_std(params["wo_t"], XA, XP, NK)
    XA = None
    for k in range(NK):
        xr = xsc()
        nc.sync.dma_start(xr[:, :], xT[k * P:(k + 1) * P, :])
        for ts in range(TS):
            xpf = sc()
            nc.vector.tensor_add(xpf[:], XP[k][:, ts * TC:(ts + 1) * TC],
                                 xr[:, 1 + ts * TC:1 + (ts + 1) * TC])
            nc.vector.tensor_copy(XP[k][:, ts * TC:(ts + 1) * TC], xpf[:])
            nc.sync.dma_start(xprime_d[k * P:(k + 1) * P, ts * TC:(ts + 1) * TC],
                              XP[k][:, ts * TC:(ts + 1) * TC])
            if ts == TS - 1:
                lc = sc((P, 1))
                nc.vector.tensor_copy(lc[:], xpf[:, TC - 1:TC])
                nc.sync.dma_start(xcol_d[:, k:k + 1], lc[:])
    nc.gpsimd.collective_compute("AllGather", AOT.bypass, replica_groups=GROUPS,
                                 ins=[xcol_d.opt()], outs=[xcgat_d.opt()])

    # ================= phase 7: LN2 + mixes =================
    XCH = const.tile([P, NK], f32, tag="xch")   # per-chunk halo cols
    nc.sync.dma_start(XCH[:], xcgat_d[0:P, :])

    H2 = [bigt() for _ in range(NK)]

    def src_x2(k):
        t = xsc()
        nc.vector.tensor_copy(t[:, 0:1], XCH[:, k:k + 1])
        nc.vector.tensor_copy(t[:, 1:1 + TL], XP[k][:, 0:TL])
        return t

    ln_pass(src_x2, LN2G, LN2B, H2, halo_mask=True)
    XP = None
    MFK = [bigt(cols=TL) for _ in range(NK)]
    MFR = [bigt(cols=TL) for _ in range(NK)]
    for k in range(NK):
        mix(MFK[k], H2[k], FMK, FMK1, k)
        mix(MFR[k], H2[k], FMR, FMR1, k)
    H2 = None

    # ================= phase 8: gate = sigmoid(mfr @ wfr) =================
    GT = [bigt(cols=TL) for _ in range(NK)]
    gemm_std(params["wfr_t"], MFR, GT, NK, act="sigmoid")
    MFR = None

    # ========== phase 9: FFN quarters: kf=relu^2(mfk@wfk); kv+=wfv^T@kf ==========
    KV = [bigt(cols=TL) for _ in range(NK)]
    for q in range(NQ):
        KF = [bigt(cols=TL) for _ in range(JQ)]
        gemm_std(params["wfk_t"], MFK, KF, JQ, act="sqrelu",
                 col_base=q * (JQ // 2) * NK * 2 * P)
        # kv partial: contract the quarter's 14 j-chunks
        gemm_std(params["wfv_t"], KF, KV, NK, accum=(q > 0), n_in=JQ, G=7,
                 col_base=q * (NK // 2) * JQ * 2 * P)
        KF = None
    MFK = None

    # ================= phase 10: y = x' + gate*kv =================
    for k in range(NK):
        for ts in range(TS):
            xp = sc(dtype=bf16)
            nc.sync.dma_start(xp[:], xprime_d[k * P:(k + 1) * P,
                                              ts * TC:(ts + 1) * TC])
            gk = sc()
            nc.vector.tensor_mul(gk[:], GT[k][:, ts * TC:(ts + 1) * TC],
                                 KV[k][:, ts * TC:(ts + 1) * TC])
            yo = sc()
            nc.vector.tensor_add(yo[:], xp[:], gk[:])
            nc.sync.dma_start(yT[k * P:(k + 1) * P, ts * TC:(ts + 1) * TC], yo[:])

    for c in reversed(ctxs):
        c.__exit__(None, None, None)


# ----------------------------------------------------------------------
# Host-side sharding / gather
# ----------------------------------------------------------------------
import ml_dtypes

_NC_CACHE = {}


def _vec_pk(v, nk=NK):
    return np.ascontiguousarray(np.asarray(v).reshape(nk, P).T.astype(np.float32))


def _make_in_maps(inputs):
    x = np.asarray(inputs["x"], np.float32)
    bf = ml_dtypes.bfloat16
    td = np.asarray(inputs["time_decay"], np.float64)
    w = np.exp(-np.exp(td))                      # [H]
    ws = w ** TC
    wspp = np.zeros((P, NP), np.float32)
    for p in range(NP):
        wspp[0:S, p] = ws[2 * p]
        wspp[S:P, p] = ws[2 * p + 1]
    wkcpp = np.zeros((P, H * 4), np.float32)
    pp = np.arange(P)
    for h in range(H):
        for j in range(4):
            wkcpp[:, h * 4 + j] = w[h] ** (TC - 1 - j * P - pp)
    def _tile_mk(W):
        """[n_in*128, n_mg*256] -> [128, n_mg*n_in*256], cols (mg, k, c)."""
        n_in = W.shape[0] // P
        n_mg = W.shape[1] // (2 * P)
        return np.ascontiguousarray(
            W.reshape(n_in, P, n_mg, 2 * P).transpose(1, 2, 0, 3)
             .reshape(P, -1).astype(bf))

    wcache = {}
    for nm, key in [("wr_t", "Wr"), ("wk_t", "Wk"), ("wo_t", "Wo"),
                    ("wfr_t", "Wfr"), ("wfk_t", "Wfk")]:
        wcache[nm] = _tile_mk(np.asarray(inputs[key], np.float32))
    Wfv = np.asarray(inputs["Wfv"], np.float32)
    wcache["wfv_t"] = np.ascontiguousarray(np.concatenate(
        [_tile_mk(Wfv[q * JQ * P:(q + 1) * JQ * P, :]) for q in range(NQ)],
        axis=1))
    Wv = np.asarray(inputs["Wv"], np.float32)
    wcache["wv_t"] = np.ascontiguousarray(
        Wv.reshape(NK, P, 4, TC).transpose(1, 2, 0, 3).reshape(P, -1).astype(bf))
    maps = []
    for c in range(8):
        b, half = c // 2, c % 2
        t0 = half * TL
        xh = np.zeros((C, 1 + TL), np.float32)
        xh[:, 1:] = x[b, t0:t0 + TL, :].T
        if half == 1:
            xh[:, 0] = x[b, t0 - 1, :]
        maps.append({
            "xT": np.ascontiguousarray(xh),
            **wcache,
            "wkcpp": wkcpp, "wspp": wspp,
            "smask": np.full((1, 1), float(half), np.float32),
            "ln1g": _vec_pk(inputs["ln1_g"]), "ln1b": _vec_pk(inputs["ln1_b"]),
            "ln2g": _vec_pk(inputs["ln2_g"]), "ln2b": _vec_pk(inputs["ln2_b"]),
            "mxk": _vec_pk(inputs["att_mix_k"]), "mxv": _vec_pk(inputs["att_mix_v"]),
            "mxr": _vec_pk(inputs["att_mix_r"]),
            "fmk": _vec_pk(inputs["ffn_mix_k"]), "fmr": _vec_pk(inputs["ffn_mix_r"]),
            "lnxg": _vec_pk(inputs["lnx_g"], NP),
            "lnxb": _vec_pk(inputs["lnx_b"], NP),
            "tdv": np.ascontiguousarray(np.asarray(inputs["time_decay"],
                                                   np.float32)[None, :]),
            "uv": np.ascontiguousarray(np.asarray(inputs["time_faaaa"],
                                                  np.float32)[None, :]),
        })
    return maps


def run_on_hw(inputs, trace=False):
    from concourse.bass_utils import run_bass_kernel_spmd
    if "nc" not in _NC_CACHE:
        _NC_CACHE["nc"] = build_nc()
    nc = _NC_CACHE["nc"]
    maps = _make_in_maps(inputs)
    res = run_bass_kernel_spmd(nc, maps, core_ids=list(range(8)), trace=trace)
    B = 4
    out = np.zeros((B, 2 * TL, C), np.float32)
    for c in range(8):
        b, half = c // 2, c % 2
        out[b, half * TL:(half + 1) * TL, :] = res.results[c]["yT"].T
    return out, res


def kernel(**inputs) -> np.ndarray:
    out, _ = run_on_hw(inputs, trace=False)
    return out
